# revision 38
# baseline (speedup 1.0000x reference)
"""MultiHeadedAttention Trainium2 Bass kernel.

Full inputs -> full outputs. Shards batch (B=8) across 8 NeuronCores,
one batch element per core. Self-contained: hardcodes all shapes.

Math per core (batch item b):
  q = Wq @ query + bq  (channels o = d*4 + h permuted to head-blocked r = h*64 + d,
                        1/sqrt(64) folded into Wq/bq)
  k = Wk @ key + bk
  Vt[n, r] = (Wv @ value + bv)^T   (computed directly in transposed layout)
  per head h: S^T[m, n] = k_h^T-chunks x q_h ; E = exp(S^T) (no max subtraction:
              scores ~ N(0,1), exp is safe in fp32)
  x'[d, n]  = sum_m Vt_aug[m, d] * E[m, n]  with Vt_aug's 65th column = ones
              so row 64 of x' = softmax denominator Z[n]
  X[r, n]   = x'[d, n] / Z[n]
  out = Wm @ X + bm   (Wm columns pre-permuted to consume head-blocked X)
"""

import numpy as np

B = 8
D = 256
N = 2048
H = 4
HD = 64
NQ = 512            # unit column width (n-quarter)
NUNITS = H * (N // NQ)   # 16 units of (head, n-quarter)
NCHUNKS = 16        # m-chunks of 128 per unit
RING = 6            # psum score ring slots of [128, NQ]
ERING = 32          # E ring slots of [128, NQ] (2 units worth)

_CACHE = {}


def _build_nc():
    import concourse.bacc as bacc
    import concourse.mybir as mybir
    import concourse.tile as tile

    F32 = mybir.dt.float32
    F32R = mybir.dt.float32r
    BF16 = mybir.dt.bfloat16
    Exp = mybir.ActivationFunctionType.Exp
    Ident = mybir.ActivationFunctionType.Identity

    nc = bacc.Bacc("TRN2", target_bir_lowering=False, debug=False, num_devices=B)

    # DRAM I/O (per-core shapes); activations+weights arrive bf16 (host
    # converts) so DMA halves and every matmul runs at the bf16 row rate.
    # q/k/v are host-pre-tiled to [half, quarter, 128, NQ] so each quarter
    # transfer is one contiguous 256KB burst.
    d_q = nc.dram_tensor("query", [2, 4, 128, NQ], BF16, kind="ExternalInput")
    d_k = nc.dram_tensor("key", [2, 4, 128, NQ], BF16, kind="ExternalInput")
    d_v = nc.dram_tensor("value", [2, 4, 128, NQ], BF16, kind="ExternalInput")
    d_wqt = nc.dram_tensor("wqt", [D, D], BF16, kind="ExternalInput")
    d_wkt = nc.dram_tensor("wkt", [D, D], BF16, kind="ExternalInput")
    d_wvt = nc.dram_tensor("wvt", [D, D], BF16, kind="ExternalInput")
    d_wmt = nc.dram_tensor("wmt", [D, D], BF16, kind="ExternalInput")
    d_bq = nc.dram_tensor("bq", [D, 1], F32, kind="ExternalInput")
    d_bk = nc.dram_tensor("bk", [D, 1], F32, kind="ExternalInput")
    d_bm = nc.dram_tensor("bm", [D, 1], F32, kind="ExternalInput")
    d_out = nc.dram_tensor("out", [D, N], BF16, kind="ExternalOutput")

    with tile.TileContext(nc) as tc:
        with (
            tc.tile_pool(name="pers", bufs=1) as pers,
            tc.tile_pool(name="epool", bufs=1) as epool,
            tc.tile_pool(name="norm", bufs=3) as normp,
            tc.tile_pool(name="mix", bufs=2, space="PSUM") as mix,
            tc.tile_pool(name="sring", bufs=1, space="PSUM") as srp,
        ):
            # ---- persistent SBUF tiles ----
            # per-quarter input tiles: dependency granularity = one DMA burst
            qin = [[pers.tile([128, NQ], BF16, tag=f"qin{i}_{t}", name=f"qin{i}_{t}")
                    for t in range(4)] for i in range(2)]
            kin = [[pers.tile([128, NQ], BF16, tag=f"kin{i}_{t}", name=f"kin{i}_{t}")
                    for t in range(4)] for i in range(2)]
            vin = [[pers.tile([128, NQ], BF16, tag=f"vin{i}_{t}", name=f"vin{i}_{t}")
                    for t in range(4)] for i in range(2)]
            wqt = [pers.tile([128, D], BF16, tag=f"wqt{i}", name=f"wqt{i}") for i in range(2)]
            wkt = [pers.tile([128, D], BF16, tag=f"wkt{i}", name=f"wkt{i}") for i in range(2)]
            wvt = [pers.tile([128, D], BF16, tag=f"wvt{i}", name=f"wvt{i}") for i in range(2)]
            wmt = [pers.tile([128, D], BF16, tag=f"wmt{i}", name=f"wmt{i}") for i in range(2)]
            bq = [pers.tile([128, 1], F32, tag=f"bq{i}", name=f"bq{i}") for i in range(2)]
            bk = [pers.tile([128, 1], F32, tag=f"bk{i}", name=f"bk{i}") for i in range(2)]
            bm = [pers.tile([128, 1], F32, tag=f"bm{i}", name=f"bm{i}") for i in range(2)]
            q_sb = [pers.tile([128, N], BF16, tag=f"q{i}", name=f"q{i}") for i in range(2)]
            # K per head in zero-padded full-height tiles: K=64 matmuls never
            # HAM-warm and run ~2.4x slow, so pad to K=128 with zero rows.
            k_sb = [pers.tile([128, N], BF16, tag=f"k{i}", name=f"k{i}") for i in range(4)]
            x_sb = [pers.tile([128, N], BF16, tag=f"x{i}", name=f"x{i}") for i in range(2)]
            o_sb = [pers.tile([128, N], BF16, tag=f"o{i}", name=f"o{i}") for i in range(2)]
            vt = pers.tile([128, NCHUNKS, H, HD + 1], BF16, tag="vt", name="vt")
            zscr = pers.tile([64, N], BF16, tag="zscr", name="zscr")
            warm = pers.tile([1, 8], F32, tag="warm", name="warm")
            # score ring: 2 ping-pong tensors (Tile deps are tensor-coarse)
            sr_ab = [srp.tile([128, 3, NQ], F32, tag=f"s{i}", name=f"s{i}")
                     for i in range(2)]
            e_ab = [epool.tile([128, 3, NQ], BF16, tag=f"E{i}", name=f"E{i}")
                    for i in range(6)]

            # ---- warm up the exp table on ACT as early as possible ----
            nc.vector.memset(warm[:], 0.0)
            nc.scalar.activation(out=warm[:], in_=warm[:], func=Exp)
            # warm the PE clock (HAM) with dummy bf16 matmuls during the
            # input DMA wait; they write a score-ring slot that the first
            # real S-triad will overwrite anyway
            wsrc = pers.tile([128, NQ], BF16, tag="wsrc", name="wsrc")
            nc.gpsimd.memset(wsrc[:], 0.25)

            def pe_warm(n):
                for i in range(n):
                    nc.tensor.matmul(sr_ab[1][:, 2, :], wsrc[:, 0:128], wsrc[:],
                                     start=True, stop=True,
                                     skip_group_check=True)

            pe_warm(13)

            # ---- input DMAs, priority order: exactly what S-triad 0 needs
            # first (wkt+bk+kin q0, then wqt+bq+qin q0), then v/k/q quarters
            # in stream-consumption order ----
            def dma_quarter(dst, dsrc, t):
                for i in range(2):
                    nc.sync.dma_start(out=dst[i][t][:], in_=dsrc[i, t])

            for i in range(2):
                rows = slice(i * 128, (i + 1) * 128)
                nc.sync.dma_start(out=wkt[i], in_=d_wkt[rows, :])
                nc.sync.dma_start(out=bk[i], in_=d_bk[rows, :])
            dma_quarter(kin, d_k, 0)
            for i in range(2):
                rows = slice(i * 128, (i + 1) * 128)
                nc.sync.dma_start(out=wqt[i], in_=d_wqt[rows, :])
                nc.sync.dma_start(out=bq[i], in_=d_bq[rows, :])
            dma_quarter(qin, d_q, 0)
            for i in range(2):
                rows = slice(i * 128, (i + 1) * 128)
                nc.sync.dma_start(out=wvt[i], in_=d_wvt[rows, :])
            # EDF order: S-side (kin/qin) quarters pace exp directly; vin
            # quarters ride the 2-triad PV slack, so they can trail
            dma_quarter(vin, d_v, 0)
            dma_quarter(kin, d_k, 1)
            dma_quarter(vin, d_v, 1)
            dma_quarter(kin, d_k, 2)
            dma_quarter(kin, d_k, 3)
            dma_quarter(qin, d_q, 1)
            dma_quarter(vin, d_v, 2)
            dma_quarter(vin, d_v, 3)
            dma_quarter(qin, d_q, 2)
            dma_quarter(qin, d_q, 3)

            def dma_wave2():
                for i in range(2):
                    rows = slice(i * 128, (i + 1) * 128)
                    nc.sync.dma_start(out=wmt[i], in_=d_wmt[rows, :])
                    nc.sync.dma_start(out=bm[i], in_=d_bm[rows, :])

            # vt ones columns; zero scratch for K-tile padding
            nc.gpsimd.memset(vt[:, :, :, HD], 1.0)
            nc.gpsimd.memset(zscr[:], 0.0)

            def zero_k_half(h):
                hp = h % 2
                nc.vector.tensor_copy(
                    out=k_sb[h][(1 - hp) * 64:(2 - hp) * 64, :], in_=zscr[:])

            zero_k_half(0)
            zero_k_half(1)

            # ---- projection helpers (512-wide rounds) ----
            def q_round(mh, nt, eng, ps=None):
                # writes q_sb[mh][:, nt*512:(nt+1)*512]
                if ps is None:
                    ps = mix.tile([128, NQ], F32, tag="mix", name="mixq")
                cols = slice(nt * NQ, (nt + 1) * NQ)
                for ih in range(2):
                    nc.tensor.matmul(
                        ps[:], wqt[ih][:, mh * 128:(mh + 1) * 128],
                        qin[ih][nt][:], start=(ih == 0), stop=(ih == 1),
                        skip_group_check=True)
                if eng == "v":
                    nc.vector.tensor_scalar_add(
                        out=q_sb[mh][:, cols], in0=ps[:], scalar1=bq[mh])
                else:
                    nc.scalar.activation(
                        out=q_sb[mh][:, cols], in_=ps[:], func=Ident,
                        bias=bq[mh], scale=1.0)

            def k_round(mh, nt, use_act, ps=None):
                # rows 0:64 -> head 2mh tile, rows 64:128 -> head 2mh+1
                if ps is None:
                    ps = mix.tile([128, NQ], F32, tag="mix", name="mixk")
                cols = slice(nt * NQ, (nt + 1) * NQ)
                for ih in range(2):
                    nc.tensor.matmul(
                        ps[:], wkt[ih][:, mh * 128:(mh + 1) * 128],
                        kin[ih][nt][:], start=(ih == 0), stop=(ih == 1),
                        skip_group_check=True)
                nc.vector.tensor_scalar_add(
                    out=k_sb[2 * mh][0:64, cols], in0=ps[0:64, :],
                    scalar1=bk[mh][0:64, :])
                if use_act:
                    nc.scalar.activation(
                        out=k_sb[2 * mh + 1][64:128, cols], in_=ps[64:128, :],
                        func=Ident, bias=bk[mh][64:128, :], scale=1.0)
                else:
                    nc.vector.tensor_scalar_add(
                        out=k_sb[2 * mh + 1][64:128, cols], in0=ps[64:128, :],
                        scalar1=bk[mh][64:128, :])

            def vt_pair(c0):
                # two V-T rounds into one [128, 512] psum tile, single evac:
                # halves the mix-buf turnarounds that serialize PE on DVE
                ps = mix.tile([128, NQ], F32, tag="mix", name="mixv2")
                for j in range(2):
                    c = c0 + j
                    qt, off = divmod(c, 4)
                    for ih in range(2):
                        nc.tensor.matmul(
                            ps[:, j * D:(j + 1) * D],
                            vin[ih][qt][:, off * 128:(off + 1) * 128],
                            wvt[ih][:],
                            start=(ih == 0), stop=(ih == 1),
                            skip_group_check=True)
                view = ps[:].rearrange("p (c h d) -> p c h d", c=2, h=H)
                nc.vector.tensor_copy(out=vt[:, c0:c0 + 2, :, 0:HD], in_=view)

            # ---- minimal upfront: just what S-triad 0 needs; everything
            # else is paced into the stream as inserts ----
            _sr_slots = [sr_ab[i][:, p, :] for i in range(2) for p in range(3)]
            _slot_i = [0]

            def next_slot():
                s = _sr_slots[_slot_i[0] % 6]
                _slot_i[0] += 1
                return s

            # upfront k-round splits its two bias-adds across DVE and the
            # still-idle ACT so S-triad 0 isn't serialized behind the DVE;
            # extra warm matmuls bridge the qin-q0 DMA wait so the PE
            # doesn't drop out of its fast p-state before S-triad 0
            k_round(0, 0, True, next_slot())
            pe_warm(5)
            q_round(0, 0, "v", next_slot())
            dma_wave2()

            # ---- late projections, inserted into the unit stream ----
            def V(c0):
                return lambda: vt_pair(c0)

            inserts = {
                # mix-pool is a FIFO ring: keep allocation order aligned
                # with DMA arrival order or a stalled tile blocks later ones
                0: [V(0), lambda: k_round(0, 1, False)],
                1: [V(2), lambda: k_round(0, 2, False)],
                2: [V(4), lambda: k_round(0, 3, False)],
                3: [V(6), lambda: q_round(0, 1, "v"), V(8)],
                4: [V(10)],
                5: [lambda: q_round(0, 2, "v"), V(12), V(14)],
                10: [lambda: zero_k_half(2)],
                12: [lambda: zero_k_half(3)],
                14: [lambda: q_round(0, 3, "v")],
                16: [lambda: k_round(1, 0, False)],
                18: [lambda: k_round(1, 1, False)],
                20: [lambda: q_round(1, 0, "v")],
                22: [lambda: q_round(1, 1, "v")],
                24: [lambda: k_round(1, 2, False)],
                26: [lambda: k_round(1, 3, False)],
                28: [lambda: q_round(1, 2, "v")],
                30: [lambda: q_round(1, 3, "v")],
                74: [lambda: o_round(0, 0)],
                75: [lambda: o_round(1, 0)],
                79: [lambda: o_round(0, 1)],
                80: [lambda: o_round(1, 1)],
                83: [lambda: o_round(0, 2)],
                84: [lambda: o_round(1, 2)],
            }

            def o_round(mh, nt):
                ps = mix.tile([128, NQ], F32, tag="mix", name="mixo")
                cols = slice(nt * NQ, (nt + 1) * NQ)
                for ih in range(2):
                    nc.tensor.matmul(
                        ps[:], wmt[ih][:, mh * 128:(mh + 1) * 128],
                        x_sb[ih][:, cols], start=(ih == 0), stop=(ih == 1),
                        skip_group_check=True)
                nc.vector.tensor_scalar_add(
                    out=o_sb[mh][:, cols], in0=ps[:], scalar1=bm[mh])
                nc.sync.dma_start(
                    out=d_out[mh * 128:(mh + 1) * 128, cols],
                    in_=o_sb[mh][:, cols])

            # ---- attention units ----
            NG = NUNITS * NCHUNKS  # 256 global chunks

            def emit_S(g):
                u, c = divmod(g, NCHUNKS)
                h, qj = divmod(u, N // NQ)
                th = h // 2
                t, p = divmod(g, 3)
                nc.tensor.matmul(
                    sr_ab[t % 2][:, p, :],
                    k_sb[h][:, c * 128:(c + 1) * 128],
                    q_sb[th][:, qj * NQ:(qj + 1) * NQ],
                    start=True, stop=True, skip_group_check=True,
                )

            def emit_exp(t, nch):
                nc.scalar.activation(
                    out=e_ab[t % 6][:, 0:nch, :],
                    in_=sr_ab[t % 2][:, 0:nch, :],
                    func=Exp,
                )

            xaccs = {}

            def emit_PV(g):
                u, c = divmod(g, NCHUNKS)
                h = u // (N // NQ)
                t, p = divmod(g, 3)
                if c == 0:
                    xaccs[u] = mix.tile([HD + 1, NQ], F32, tag="mix", name="xa")
                nc.tensor.matmul(
                    xaccs[u][:],
                    vt[:, c, h, :],
                    e_ab[t % 6][:, p, :],
                    start=(c == 0), stop=(c == NCHUNKS - 1),
                    skip_group_check=True,
                )

            def emit_norm(u):
                h, qj = divmod(u, N // NQ)
                th, hp = divmod(h, 2)
                xa = xaccs.pop(u)
                if u == NUNITS - 1:
                    # final unit is on the critical tail: pipeline the norm
                    # in halves and stage zrow via the now-idle ACT
                    NH = NQ // 2
                    for half in range(2):
                        cs = slice(half * NH, (half + 1) * NH)
                        ocs = slice(qj * NQ + half * NH,
                                    qj * NQ + (half + 1) * NH)
                        zrow = normp.tile([1, NH], F32, tag="zrow",
                                          name="zrowh")
                        nc.scalar.copy(out=zrow[:], in_=xa[HD:HD + 1, cs])
                        zrec = normp.tile([1, NH], F32, tag="zrec",
                                          name="zrech")
                        nc.vector.reciprocal_approx_fast(out=zrec[:],
                                                         in_=zrow[:])
                        zb = normp.tile([64, NH], F32, tag="zb", name="zbh")
                        nc.gpsimd.partition_broadcast(zb[:], zrec[:])
                        nc.vector.tensor_tensor(
                            out=x_sb[th][hp * 64:(hp + 1) * 64, ocs],
                            in0=xa[0:HD, cs],
                            in1=zb[:],
                            op=mybir.AluOpType.mult,
                        )
                    return
                zrow = normp.tile([1, NQ], F32, tag="zrow", name="zrow")
                nc.vector.tensor_copy(out=zrow[:], in_=xa[HD:HD + 1, :])
                zrec = normp.tile([1, NQ], F32, tag="zrec", name="zrec")
                nc.vector.reciprocal_approx_fast(out=zrec[:], in_=zrow[:])
                zb = normp.tile([64, NQ], F32, tag="zb", name="zb")
                nc.gpsimd.partition_broadcast(zb[:], zrec[:])
                nc.vector.tensor_tensor(
                    out=x_sb[th][hp * 64:(hp + 1) * 64, qj * NQ:(qj + 1) * NQ],
                    in0=xa[0:HD, :],
                    in1=zb[:],
                    op=mybir.AluOpType.mult,
                )

            def emit_pv_triad(chunks):
                for g in chunks:
                    emit_PV(g)
                    if g % NCHUNKS == NCHUNKS - 1:
                        emit_norm(g // NCHUNKS)

            # emission per triad T: S(T); PV(T-2); exp(T)
            triads = [list(range(t * 3, min(t * 3 + 3, NG)))
                      for t in range((NG + 2) // 3)]
            for t, chunks in enumerate(triads):
                for g in chunks:
                    emit_S(g)
                if t >= 2:
                    emit_pv_triad(triads[t - 2])
                for fn in inserts.get(t, ()):
                    fn()
                emit_exp(t, len(chunks))
            emit_pv_triad(triads[-2])
            emit_pv_triad(triads[-1])

            # ---- tail output rounds: pre-accumulate the x_sb[0] half of
            # o(0,3) before the final norm lands, keep the PE p-state warm
            # across the norm chain, then finish ----
            cols3 = slice(3 * NQ, 4 * NQ)
            ps03 = mix.tile([128, NQ], F32, tag="mix", name="mixo3")
            nc.tensor.matmul(
                ps03[:], wmt[0][:, 0:128], x_sb[0][:, cols3],
                start=True, stop=False, skip_group_check=True)
            pe_warm(10)
            nc.tensor.matmul(
                ps03[:], wmt[1][:, 0:128], x_sb[1][:, cols3],
                start=False, stop=True, skip_group_check=True)
            nc.vector.tensor_scalar_add(
                out=o_sb[0][:, cols3], in0=ps03[:], scalar1=bm[0])
            nc.sync.dma_start(out=d_out[0:128, cols3], in_=o_sb[0][:, cols3])
            o_round(1, 3)

    nc.finalize()
    return nc


def _get_nc():
    if "nc" not in _CACHE:
        _CACHE["nc"] = _build_nc()
    return _CACHE["nc"]


def _prep_host(Wq, bq, Wk, bk, Wv, bv, Wm, bm):
    import ml_dtypes

    r = np.arange(D)
    perm = (r % HD) * H + (r // HD)  # head-blocked row r -> original channel o
    s = np.float32(1.0 / np.sqrt(HD))
    bf16 = ml_dtypes.bfloat16
    f32 = np.float32
    wqt = np.ascontiguousarray((Wq[perm, :] * s).T, dtype=bf16)
    bq_p = np.ascontiguousarray((bq[perm] * s)[:, None], dtype=f32)
    wkt = np.ascontiguousarray(Wk[perm, :].T, dtype=bf16)
    bk_p = np.ascontiguousarray(bk[perm][:, None], dtype=f32)
    wvt = np.ascontiguousarray(Wv[perm, :].T, dtype=bf16)
    wmt = np.ascontiguousarray(Wm[:, perm].T, dtype=bf16)
    # V-bias folds into the output projection bias: X = X0 + bv (per row),
    # so out = Wm_hb @ X0 + (bm + Wm_hb @ bv_hb)
    bm_p = np.ascontiguousarray(
        (bm + Wm[:, perm] @ bv[perm])[:, None], dtype=f32)
    return dict(wqt=wqt, bq=bq_p, wkt=wkt, bk=bk_p, wvt=wvt,
                wmt=wmt, bm=bm_p)


def _run(inputs, trace=False):
    import ml_dtypes
    from concourse.bass_utils import run_bass_kernel_spmd

    bf16 = ml_dtypes.bfloat16

    def _tile_qkv(x):
        # [B, D, N] f32 -> [B, 2, 4, 128, NQ] bf16, each quarter contiguous
        return np.ascontiguousarray(
            np.asarray(x, dtype=np.float32)
            .reshape(B, 2, 128, 4, NQ).transpose(0, 1, 3, 2, 4).astype(bf16))

    query = _tile_qkv(inputs["query"])
    key = _tile_qkv(inputs["key"])
    value = _tile_qkv(inputs["value"])
    w = _prep_host(
        np.asarray(inputs["Wq"], np.float32), np.asarray(inputs["bq"], np.float32),
        np.asarray(inputs["Wk"], np.float32), np.asarray(inputs["bk"], np.float32),
        np.asarray(inputs["Wv"], np.float32), np.asarray(inputs["bv"], np.float32),
        np.asarray(inputs["Wm"], np.float32), np.asarray(inputs["bm"], np.float32),
    )
    in_maps = []
    for b in range(B):
        m = dict(w)
        m["query"] = np.ascontiguousarray(query[b])
        m["key"] = np.ascontiguousarray(key[b])
        m["value"] = np.ascontiguousarray(value[b])
        in_maps.append(m)
    nc = _get_nc()
    res = run_bass_kernel_spmd(nc, in_maps, core_ids=list(range(B)), trace=trace)
    out = np.stack([np.asarray(r["out"], dtype=np.float32) for r in res.results],
                   axis=0)
    return out, res


def kernel(**inputs):
    out, _ = _run(inputs, trace=False)
    return out


if __name__ == "__main__":
    rng = np.random.default_rng(0)
    s = 1.0 / np.sqrt(D)
    inputs = {
        "query": rng.standard_normal((B, D, N), dtype=np.float32),
        "key": rng.standard_normal((B, D, N), dtype=np.float32),
        "value": rng.standard_normal((B, D, N), dtype=np.float32),
        "Wq": rng.standard_normal((D, D), dtype=np.float32) * s,
        "bq": rng.standard_normal((D,), dtype=np.float32) * 0.01,
        "Wk": rng.standard_normal((D, D), dtype=np.float32) * s,
        "bk": rng.standard_normal((D,), dtype=np.float32) * 0.01,
        "Wv": rng.standard_normal((D, D), dtype=np.float32) * s,
        "bv": rng.standard_normal((D,), dtype=np.float32) * 0.01,
        "Wm": rng.standard_normal((D, D), dtype=np.float32) * s,
        "bm": rng.standard_normal((D,), dtype=np.float32) * 0.01,
    }
    out = kernel(**inputs)
    # numpy reference
    def proj(x, W, b):
        return np.einsum("oi,bin->bon", W, x) + b[None, :, None]
    q = proj(inputs["query"], inputs["Wq"], inputs["bq"]).reshape(B, HD, H, N)
    k = proj(inputs["key"], inputs["Wk"], inputs["bk"]).reshape(B, HD, H, N)
    v = proj(inputs["value"], inputs["Wv"], inputs["bv"]).reshape(B, HD, H, N)
    sc = np.einsum("bdhn,bdhm->bhnm", q, k) / np.sqrt(HD)
    sc = sc - sc.max(axis=-1, keepdims=True)
    p = np.exp(sc)
    p /= p.sum(axis=-1, keepdims=True)
    x = np.einsum("bhnm,bdhm->bdhn", p, v).reshape(B, D, N)
    ref = proj(x, inputs["Wm"], inputs["bm"])
    err = np.abs(out - ref)
    scale = np.abs(ref).max()
    print("abs err max:", err.max(), "scaled:", err.max() / scale)
    rel = np.linalg.norm(out - ref) / np.linalg.norm(ref)
    print("fro rel err:", rel)



# revision 41
# speedup vs baseline: 1.0076x; 1.0076x over previous
"""MultiHeadedAttention Trainium2 Bass kernel.

Full inputs -> full outputs. Shards batch (B=8) across 8 NeuronCores,
one batch element per core. Self-contained: hardcodes all shapes.

Math per core (batch item b):
  q = Wq @ query + bq  (channels o = d*4 + h permuted to head-blocked r = h*64 + d,
                        1/sqrt(64) folded into Wq/bq)
  k = Wk @ key + bk
  Vt[n, r] = (Wv @ value + bv)^T   (computed directly in transposed layout)
  per head h: S^T[m, n] = k_h^T-chunks x q_h ; E = exp(S^T) (no max subtraction:
              scores ~ N(0,1), exp is safe in fp32)
  x'[d, n]  = sum_m Vt_aug[m, d] * E[m, n]  with Vt_aug's 65th column = ones
              so row 64 of x' = softmax denominator Z[n]
  X[r, n]   = x'[d, n] / Z[n]
  out = Wm @ X + bm   (Wm columns pre-permuted to consume head-blocked X)
"""

import numpy as np

B = 8
D = 256
N = 2048
H = 4
HD = 64
NQ = 512            # unit column width (n-quarter)
NUNITS = H * (N // NQ)   # 16 units of (head, n-quarter)
NCHUNKS = 16        # m-chunks of 128 per unit
RING = 6            # psum score ring slots of [128, NQ]
ERING = 32          # E ring slots of [128, NQ] (2 units worth)

_CACHE = {}


def _build_nc():
    import concourse.bacc as bacc
    import concourse.mybir as mybir
    import concourse.tile as tile

    F32 = mybir.dt.float32
    F32R = mybir.dt.float32r
    BF16 = mybir.dt.bfloat16
    Exp = mybir.ActivationFunctionType.Exp
    Ident = mybir.ActivationFunctionType.Identity

    nc = bacc.Bacc("TRN2", target_bir_lowering=False, debug=False, num_devices=B)

    # DRAM I/O (per-core shapes); activations+weights arrive bf16 (host
    # converts) so DMA halves and every matmul runs at the bf16 row rate.
    # q/k/v are host-pre-tiled to [half, quarter, 128, NQ] so each quarter
    # transfer is one contiguous 256KB burst.
    d_q = nc.dram_tensor("query", [2, 4, 128, NQ], BF16, kind="ExternalInput")
    d_k = nc.dram_tensor("key", [2, 4, 128, NQ], BF16, kind="ExternalInput")
    d_v = nc.dram_tensor("value", [2, 4, 128, NQ], BF16, kind="ExternalInput")
    d_wqt = nc.dram_tensor("wqt", [D, D], BF16, kind="ExternalInput")
    d_wkt = nc.dram_tensor("wkt", [D, D], BF16, kind="ExternalInput")
    d_wvt = nc.dram_tensor("wvt", [D, D], BF16, kind="ExternalInput")
    d_wmt = nc.dram_tensor("wmt", [D, D], BF16, kind="ExternalInput")
    d_bq = nc.dram_tensor("bq", [D, 1], F32, kind="ExternalInput")
    d_bk = nc.dram_tensor("bk", [D, 1], F32, kind="ExternalInput")
    d_bm = nc.dram_tensor("bm", [D, 1], F32, kind="ExternalInput")
    d_out = nc.dram_tensor("out", [D, N], BF16, kind="ExternalOutput")

    with tile.TileContext(nc) as tc:
        with (
            tc.tile_pool(name="pers", bufs=1) as pers,
            tc.tile_pool(name="epool", bufs=1) as epool,
            tc.tile_pool(name="norm", bufs=3) as normp,
            tc.tile_pool(name="mix", bufs=2, space="PSUM") as mix,
            tc.tile_pool(name="sring", bufs=1, space="PSUM") as srp,
        ):
            # ---- persistent SBUF tiles ----
            # per-quarter input tiles: dependency granularity = one DMA burst
            qin = [[pers.tile([128, NQ], BF16, tag=f"qin{i}_{t}", name=f"qin{i}_{t}")
                    for t in range(4)] for i in range(2)]
            kin = [[pers.tile([128, NQ], BF16, tag=f"kin{i}_{t}", name=f"kin{i}_{t}")
                    for t in range(4)] for i in range(2)]
            vin = [[pers.tile([128, NQ], BF16, tag=f"vin{i}_{t}", name=f"vin{i}_{t}")
                    for t in range(4)] for i in range(2)]
            wqt = [pers.tile([128, D], BF16, tag=f"wqt{i}", name=f"wqt{i}") for i in range(2)]
            wkt = [pers.tile([128, D], BF16, tag=f"wkt{i}", name=f"wkt{i}") for i in range(2)]
            wvt = [pers.tile([128, D], BF16, tag=f"wvt{i}", name=f"wvt{i}") for i in range(2)]
            wmt = [pers.tile([128, D], BF16, tag=f"wmt{i}", name=f"wmt{i}") for i in range(2)]
            bq = [pers.tile([128, 1], F32, tag=f"bq{i}", name=f"bq{i}") for i in range(2)]
            bk = [pers.tile([128, 1], F32, tag=f"bk{i}", name=f"bk{i}") for i in range(2)]
            bm = [pers.tile([128, 1], F32, tag=f"bm{i}", name=f"bm{i}") for i in range(2)]
            q_sb = [pers.tile([128, N], BF16, tag=f"q{i}", name=f"q{i}") for i in range(2)]
            # K per head in zero-padded full-height tiles: K=64 matmuls never
            # HAM-warm and run ~2.4x slow, so pad to K=128 with zero rows.
            k_sb = [pers.tile([128, N], BF16, tag=f"k{i}", name=f"k{i}") for i in range(4)]
            x_sb = [pers.tile([128, N], BF16, tag=f"x{i}", name=f"x{i}") for i in range(2)]
            o_sb = [pers.tile([128, N], BF16, tag=f"o{i}", name=f"o{i}") for i in range(2)]
            vt = pers.tile([128, NCHUNKS, H, HD + 1], BF16, tag="vt", name="vt")
            zscr = pers.tile([64, N], BF16, tag="zscr", name="zscr")
            warm = pers.tile([1, 8], F32, tag="warm", name="warm")
            # score ring: 2 ping-pong tensors (Tile deps are tensor-coarse)
            sr_ab = [srp.tile([128, 3, NQ], F32, tag=f"s{i}", name=f"s{i}")
                     for i in range(2)]
            e_ab = [epool.tile([128, 3, NQ], BF16, tag=f"E{i}", name=f"E{i}")
                    for i in range(6)]

            # ---- warm up the exp table on ACT as early as possible ----
            nc.vector.memset(warm[:], 0.0)
            nc.scalar.activation(out=warm[:], in_=warm[:], func=Exp)
            # warm the PE clock (HAM) with dummy bf16 matmuls during the
            # input DMA wait; they write a score-ring slot that the first
            # real S-triad will overwrite anyway
            wsrc = pers.tile([128, NQ], BF16, tag="wsrc", name="wsrc")
            nc.gpsimd.memset(wsrc[:], 0.25)

            def pe_warm(n):
                for i in range(n):
                    nc.tensor.matmul(sr_ab[1][:, 2, :], wsrc[:, 0:128], wsrc[:],
                                     start=True, stop=True,
                                     skip_group_check=True)

            # deliberately long warm-up: S-triad 0 should start only once
            # enough input has landed that the fill never stalls the
            # (in-order) PE queue — a stall drops the p-state and slows
            # every fill matmul ~2x
            pe_warm(48)

            # ---- input DMAs, priority order: exactly what S-triad 0 needs
            # first (wkt+bk+kin q0, then wqt+bq+qin q0), then v/k/q quarters
            # in stream-consumption order ----
            def dma_quarter(dst, dsrc, t):
                for i in range(2):
                    nc.sync.dma_start(out=dst[i][t][:], in_=dsrc[i, t])

            for i in range(2):
                rows = slice(i * 128, (i + 1) * 128)
                nc.sync.dma_start(out=wkt[i], in_=d_wkt[rows, :])
                nc.sync.dma_start(out=bk[i], in_=d_bk[rows, :])
            dma_quarter(kin, d_k, 0)
            for i in range(2):
                rows = slice(i * 128, (i + 1) * 128)
                nc.sync.dma_start(out=wqt[i], in_=d_wqt[rows, :])
                nc.sync.dma_start(out=bq[i], in_=d_bq[rows, :])
            dma_quarter(qin, d_q, 0)
            for i in range(2):
                rows = slice(i * 128, (i + 1) * 128)
                nc.sync.dma_start(out=wvt[i], in_=d_wvt[rows, :])
            # EDF order: S-side (kin/qin) quarters pace exp directly; vin
            # quarters ride the 2-triad PV slack, so they can trail
            dma_quarter(vin, d_v, 0)
            dma_quarter(kin, d_k, 1)
            dma_quarter(vin, d_v, 1)
            dma_quarter(kin, d_k, 2)
            dma_quarter(qin, d_q, 1)
            dma_quarter(vin, d_v, 2)
            dma_quarter(kin, d_k, 3)
            dma_quarter(qin, d_q, 2)
            dma_quarter(vin, d_v, 3)
            dma_quarter(qin, d_q, 3)

            def dma_wave2():
                for i in range(2):
                    rows = slice(i * 128, (i + 1) * 128)
                    nc.sync.dma_start(out=wmt[i], in_=d_wmt[rows, :])
                    nc.sync.dma_start(out=bm[i], in_=d_bm[rows, :])

            # vt ones columns; zero scratch for K-tile padding
            nc.gpsimd.memset(vt[:, :, :, HD], 1.0)
            nc.gpsimd.memset(zscr[:], 0.0)

            def zero_k_half(h):
                hp = h % 2
                nc.vector.tensor_copy(
                    out=k_sb[h][(1 - hp) * 64:(2 - hp) * 64, :], in_=zscr[:])

            zero_k_half(0)
            zero_k_half(1)

            # ---- projection helpers (512-wide rounds) ----
            def q_round(mh, nt, eng, ps=None):
                # writes q_sb[mh][:, nt*512:(nt+1)*512]
                if ps is None:
                    ps = mix.tile([128, NQ], F32, tag="mix", name="mixq")
                cols = slice(nt * NQ, (nt + 1) * NQ)
                for ih in range(2):
                    nc.tensor.matmul(
                        ps[:], wqt[ih][:, mh * 128:(mh + 1) * 128],
                        qin[ih][nt][:], start=(ih == 0), stop=(ih == 1),
                        skip_group_check=True)
                if eng == "v":
                    nc.vector.tensor_scalar_add(
                        out=q_sb[mh][:, cols], in0=ps[:], scalar1=bq[mh])
                else:
                    nc.scalar.activation(
                        out=q_sb[mh][:, cols], in_=ps[:], func=Ident,
                        bias=bq[mh], scale=1.0)

            def k_round(mh, nt, use_act, ps=None):
                # rows 0:64 -> head 2mh tile, rows 64:128 -> head 2mh+1
                if ps is None:
                    ps = mix.tile([128, NQ], F32, tag="mix", name="mixk")
                cols = slice(nt * NQ, (nt + 1) * NQ)
                for ih in range(2):
                    nc.tensor.matmul(
                        ps[:], wkt[ih][:, mh * 128:(mh + 1) * 128],
                        kin[ih][nt][:], start=(ih == 0), stop=(ih == 1),
                        skip_group_check=True)
                nc.vector.tensor_scalar_add(
                    out=k_sb[2 * mh][0:64, cols], in0=ps[0:64, :],
                    scalar1=bk[mh][0:64, :])
                if use_act:
                    nc.scalar.activation(
                        out=k_sb[2 * mh + 1][64:128, cols], in_=ps[64:128, :],
                        func=Ident, bias=bk[mh][64:128, :], scale=1.0)
                else:
                    nc.vector.tensor_scalar_add(
                        out=k_sb[2 * mh + 1][64:128, cols], in0=ps[64:128, :],
                        scalar1=bk[mh][64:128, :])

            def vt_pair(c0):
                # two V-T rounds into one [128, 512] psum tile, single evac:
                # halves the mix-buf turnarounds that serialize PE on DVE
                ps = mix.tile([128, NQ], F32, tag="mix", name="mixv2")
                for j in range(2):
                    c = c0 + j
                    qt, off = divmod(c, 4)
                    for ih in range(2):
                        nc.tensor.matmul(
                            ps[:, j * D:(j + 1) * D],
                            vin[ih][qt][:, off * 128:(off + 1) * 128],
                            wvt[ih][:],
                            start=(ih == 0), stop=(ih == 1),
                            skip_group_check=True)
                view = ps[:].rearrange("p (c h d) -> p c h d", c=2, h=H)
                nc.vector.tensor_copy(out=vt[:, c0:c0 + 2, :, 0:HD], in_=view)

            # ---- minimal upfront: just what S-triad 0 needs; everything
            # else is paced into the stream as inserts ----
            _sr_slots = [sr_ab[i][:, p, :] for i in range(2) for p in range(3)]
            _slot_i = [0]

            def next_slot():
                s = _sr_slots[_slot_i[0] % 6]
                _slot_i[0] += 1
                return s

            # upfront k-round splits its two bias-adds across DVE and the
            # still-idle ACT so S-triad 0 isn't serialized behind the DVE;
            # extra warm matmuls bridge the qin-q0 DMA wait so the PE
            # doesn't drop out of its fast p-state before S-triad 0
            k_round(0, 0, True, next_slot())
            q_round(0, 0, "v", next_slot())
            dma_wave2()

            # ---- late projections, inserted into the unit stream ----
            def V(c0):
                return lambda: vt_pair(c0)

            inserts = {
                # mix-pool is a FIFO ring: keep allocation order aligned
                # with DMA arrival order or a stalled tile blocks later ones
                0: [V(0), lambda: k_round(0, 1, False)],
                1: [V(2), lambda: k_round(0, 2, False)],
                2: [V(4), lambda: k_round(0, 3, False)],
                3: [V(6), lambda: q_round(0, 1, "v"), V(8)],
                4: [V(10)],
                5: [lambda: q_round(0, 2, "v"), V(12), V(14)],
                10: [lambda: zero_k_half(2)],
                12: [lambda: zero_k_half(3)],
                14: [lambda: q_round(0, 3, "v")],
                16: [lambda: k_round(1, 0, False)],
                18: [lambda: k_round(1, 1, False)],
                20: [lambda: q_round(1, 0, "v")],
                22: [lambda: q_round(1, 1, "v")],
                24: [lambda: k_round(1, 2, False)],
                26: [lambda: k_round(1, 3, False)],
                28: [lambda: q_round(1, 2, "v")],
                30: [lambda: q_round(1, 3, "v")],
                74: [lambda: o_round(0, 0)],
                75: [lambda: o_round(1, 0)],
                79: [lambda: o_round(0, 1)],
                80: [lambda: o_round(1, 1)],
                83: [lambda: o_round(0, 2)],
                84: [lambda: o_round(1, 2)],
            }

            def o_round(mh, nt):
                ps = mix.tile([128, NQ], F32, tag="mix", name="mixo")
                cols = slice(nt * NQ, (nt + 1) * NQ)
                for ih in range(2):
                    nc.tensor.matmul(
                        ps[:], wmt[ih][:, mh * 128:(mh + 1) * 128],
                        x_sb[ih][:, cols], start=(ih == 0), stop=(ih == 1),
                        skip_group_check=True)
                nc.vector.tensor_scalar_add(
                    out=o_sb[mh][:, cols], in0=ps[:], scalar1=bm[mh])
                nc.sync.dma_start(
                    out=d_out[mh * 128:(mh + 1) * 128, cols],
                    in_=o_sb[mh][:, cols])

            # ---- attention units ----
            NG = NUNITS * NCHUNKS  # 256 global chunks

            def emit_S(g):
                u, c = divmod(g, NCHUNKS)
                h, qj = divmod(u, N // NQ)
                th = h // 2
                t, p = divmod(g, 3)
                nc.tensor.matmul(
                    sr_ab[t % 2][:, p, :],
                    k_sb[h][:, c * 128:(c + 1) * 128],
                    q_sb[th][:, qj * NQ:(qj + 1) * NQ],
                    start=True, stop=True, skip_group_check=True,
                )

            def emit_exp(t, nch):
                nc.scalar.activation(
                    out=e_ab[t % 6][:, 0:nch, :],
                    in_=sr_ab[t % 2][:, 0:nch, :],
                    func=Exp,
                )

            xaccs = {}

            def emit_PV(g):
                u, c = divmod(g, NCHUNKS)
                h = u // (N // NQ)
                t, p = divmod(g, 3)
                if c == 0:
                    xaccs[u] = mix.tile([HD + 1, NQ], F32, tag="mix", name="xa")
                nc.tensor.matmul(
                    xaccs[u][:],
                    vt[:, c, h, :],
                    e_ab[t % 6][:, p, :],
                    start=(c == 0), stop=(c == NCHUNKS - 1),
                    skip_group_check=True,
                )

            def emit_norm(u):
                h, qj = divmod(u, N // NQ)
                th, hp = divmod(h, 2)
                xa = xaccs.pop(u)
                if u == NUNITS - 1:
                    # final unit is on the critical tail: pipeline the norm
                    # in halves and stage zrow via the now-idle ACT
                    NH = NQ // 2
                    for half in range(2):
                        cs = slice(half * NH, (half + 1) * NH)
                        ocs = slice(qj * NQ + half * NH,
                                    qj * NQ + (half + 1) * NH)
                        zrow = normp.tile([1, NH], F32, tag="zrow",
                                          name="zrowh")
                        nc.scalar.copy(out=zrow[:], in_=xa[HD:HD + 1, cs])
                        zrec = normp.tile([1, NH], F32, tag="zrec",
                                          name="zrech")
                        nc.vector.reciprocal_approx_fast(out=zrec[:],
                                                         in_=zrow[:])
                        zb = normp.tile([64, NH], F32, tag="zb", name="zbh")
                        nc.gpsimd.partition_broadcast(zb[:], zrec[:])
                        nc.vector.tensor_tensor(
                            out=x_sb[th][hp * 64:(hp + 1) * 64, ocs],
                            in0=xa[0:HD, cs],
                            in1=zb[:],
                            op=mybir.AluOpType.mult,
                        )
                    return
                zrow = normp.tile([1, NQ], F32, tag="zrow", name="zrow")
                nc.vector.tensor_copy(out=zrow[:], in_=xa[HD:HD + 1, :])
                zrec = normp.tile([1, NQ], F32, tag="zrec", name="zrec")
                nc.vector.reciprocal_approx_fast(out=zrec[:], in_=zrow[:])
                zb = normp.tile([64, NQ], F32, tag="zb", name="zb")
                nc.gpsimd.partition_broadcast(zb[:], zrec[:])
                nc.vector.tensor_tensor(
                    out=x_sb[th][hp * 64:(hp + 1) * 64, qj * NQ:(qj + 1) * NQ],
                    in0=xa[0:HD, :],
                    in1=zb[:],
                    op=mybir.AluOpType.mult,
                )

            def emit_pv_triad(chunks):
                for g in chunks:
                    emit_PV(g)
                    if g % NCHUNKS == NCHUNKS - 1:
                        emit_norm(g // NCHUNKS)

            # emission per triad T: S(T); PV(T-2); exp(T)
            triads = [list(range(t * 3, min(t * 3 + 3, NG)))
                      for t in range((NG + 2) // 3)]
            for t, chunks in enumerate(triads):
                for g in chunks:
                    emit_S(g)
                if t >= 2:
                    emit_pv_triad(triads[t - 2])
                for fn in inserts.get(t, ()):
                    fn()
                emit_exp(t, len(chunks))
            emit_pv_triad(triads[-2])
            emit_pv_triad(triads[-1])

            # ---- tail output rounds: pre-accumulate the x_sb[0] half of
            # o(0,3) before the final norm lands, keep the PE p-state warm
            # across the norm chain, then finish ----
            cols3 = slice(3 * NQ, 4 * NQ)
            ps03 = mix.tile([128, NQ], F32, tag="mix", name="mixo3")
            nc.tensor.matmul(
                ps03[:], wmt[0][:, 0:128], x_sb[0][:, cols3],
                start=True, stop=False, skip_group_check=True)
            pe_warm(10)
            nc.tensor.matmul(
                ps03[:], wmt[1][:, 0:128], x_sb[1][:, cols3],
                start=False, stop=True, skip_group_check=True)
            nc.vector.tensor_scalar_add(
                out=o_sb[0][:, cols3], in0=ps03[:], scalar1=bm[0])
            nc.sync.dma_start(out=d_out[0:128, cols3], in_=o_sb[0][:, cols3])
            o_round(1, 3)

    nc.finalize()
    return nc


def _get_nc():
    if "nc" not in _CACHE:
        _CACHE["nc"] = _build_nc()
    return _CACHE["nc"]


def _prep_host(Wq, bq, Wk, bk, Wv, bv, Wm, bm):
    import ml_dtypes

    r = np.arange(D)
    perm = (r % HD) * H + (r // HD)  # head-blocked row r -> original channel o
    s = np.float32(1.0 / np.sqrt(HD))
    bf16 = ml_dtypes.bfloat16
    f32 = np.float32
    wqt = np.ascontiguousarray((Wq[perm, :] * s).T, dtype=bf16)
    bq_p = np.ascontiguousarray((bq[perm] * s)[:, None], dtype=f32)
    wkt = np.ascontiguousarray(Wk[perm, :].T, dtype=bf16)
    bk_p = np.ascontiguousarray(bk[perm][:, None], dtype=f32)
    wvt = np.ascontiguousarray(Wv[perm, :].T, dtype=bf16)
    wmt = np.ascontiguousarray(Wm[:, perm].T, dtype=bf16)
    # V-bias folds into the output projection bias: X = X0 + bv (per row),
    # so out = Wm_hb @ X0 + (bm + Wm_hb @ bv_hb)
    bm_p = np.ascontiguousarray(
        (bm + Wm[:, perm] @ bv[perm])[:, None], dtype=f32)
    return dict(wqt=wqt, bq=bq_p, wkt=wkt, bk=bk_p, wvt=wvt,
                wmt=wmt, bm=bm_p)


def _run(inputs, trace=False):
    import ml_dtypes
    from concourse.bass_utils import run_bass_kernel_spmd

    bf16 = ml_dtypes.bfloat16

    def _tile_qkv(x):
        # [B, D, N] f32 -> [B, 2, 4, 128, NQ] bf16, each quarter contiguous
        return np.ascontiguousarray(
            np.asarray(x, dtype=np.float32)
            .reshape(B, 2, 128, 4, NQ).transpose(0, 1, 3, 2, 4).astype(bf16))

    query = _tile_qkv(inputs["query"])
    key = _tile_qkv(inputs["key"])
    value = _tile_qkv(inputs["value"])
    w = _prep_host(
        np.asarray(inputs["Wq"], np.float32), np.asarray(inputs["bq"], np.float32),
        np.asarray(inputs["Wk"], np.float32), np.asarray(inputs["bk"], np.float32),
        np.asarray(inputs["Wv"], np.float32), np.asarray(inputs["bv"], np.float32),
        np.asarray(inputs["Wm"], np.float32), np.asarray(inputs["bm"], np.float32),
    )
    in_maps = []
    for b in range(B):
        m = dict(w)
        m["query"] = np.ascontiguousarray(query[b])
        m["key"] = np.ascontiguousarray(key[b])
        m["value"] = np.ascontiguousarray(value[b])
        in_maps.append(m)
    nc = _get_nc()
    res = run_bass_kernel_spmd(nc, in_maps, core_ids=list(range(B)), trace=trace)
    out = np.stack([np.asarray(r["out"], dtype=np.float32) for r in res.results],
                   axis=0)
    return out, res


def kernel(**inputs):
    out, _ = _run(inputs, trace=False)
    return out


if __name__ == "__main__":
    rng = np.random.default_rng(0)
    s = 1.0 / np.sqrt(D)
    inputs = {
        "query": rng.standard_normal((B, D, N), dtype=np.float32),
        "key": rng.standard_normal((B, D, N), dtype=np.float32),
        "value": rng.standard_normal((B, D, N), dtype=np.float32),
        "Wq": rng.standard_normal((D, D), dtype=np.float32) * s,
        "bq": rng.standard_normal((D,), dtype=np.float32) * 0.01,
        "Wk": rng.standard_normal((D, D), dtype=np.float32) * s,
        "bk": rng.standard_normal((D,), dtype=np.float32) * 0.01,
        "Wv": rng.standard_normal((D, D), dtype=np.float32) * s,
        "bv": rng.standard_normal((D,), dtype=np.float32) * 0.01,
        "Wm": rng.standard_normal((D, D), dtype=np.float32) * s,
        "bm": rng.standard_normal((D,), dtype=np.float32) * 0.01,
    }
    out = kernel(**inputs)
    # numpy reference
    def proj(x, W, b):
        return np.einsum("oi,bin->bon", W, x) + b[None, :, None]
    q = proj(inputs["query"], inputs["Wq"], inputs["bq"]).reshape(B, HD, H, N)
    k = proj(inputs["key"], inputs["Wk"], inputs["bk"]).reshape(B, HD, H, N)
    v = proj(inputs["value"], inputs["Wv"], inputs["bv"]).reshape(B, HD, H, N)
    sc = np.einsum("bdhn,bdhm->bhnm", q, k) / np.sqrt(HD)
    sc = sc - sc.max(axis=-1, keepdims=True)
    p = np.exp(sc)
    p /= p.sum(axis=-1, keepdims=True)
    x = np.einsum("bhnm,bdhm->bdhn", p, v).reshape(B, D, N)
    ref = proj(x, inputs["Wm"], inputs["bm"])
    err = np.abs(out - ref)
    scale = np.abs(ref).max()
    print("abs err max:", err.max(), "scaled:", err.max() / scale)
    rel = np.linalg.norm(out - ref) / np.linalg.norm(ref)
    print("fro rel err:", rel)

